# revision 23
# baseline (speedup 1.0000x reference)
"""CoLaLoLa (gnn_message_passing) Trainium2 Bass kernel.

Strategy
--------
Pure data parallel over 8 NeuronCores: batch B=2048 -> 256 rows/core.

Math restructure (avoids the [B,128,128,4] pairwise tensor entirely):
  distances[b,n,m] = masses[b,n] + masses[b,m] - 2*sum_i M_i cv[b,n,i] cv[b,m,i]
  => weighted_d[b,n] = masses[b,n]*rowsum_w[n] + (w_dist @ masses[b])[n]
                       - sum_i cv[b,n,i] * u'_i[b,n],   u'_i = 2 M_i w_dist @ cv_i

Everything is computed feature-major ([feature_partition, batch_free]) so all
contractions are TensorE matmuls with host-prefused stationary weights
(combo = [eye(50); w_combo]):
  A_cv = combo.T, A_un/A_up = -/+(2*w_dist @ combo).T, A_e/A_p likewise.
BatchNorm needs global batch stats -> two launches with a tiny host reduction
in between; the BN scale/shift is folded into W1 on the host between launches.

Perf notes: every dma_start costs ~650ns of serialized sequencer issue, so all
weights are packed into one blob per launch (single DMA); component pairs are
processed as [128,512] tiles to halve instruction counts; elementwise work is
spread over ACT/DVE/GPSIMD; matmul operands can be bitcast to float32r.
"""
import sys

sys.path.insert(0, "/opt/trn_rl_repo")

from contextlib import ExitStack

import numpy as np

import concourse.bass as bass
import concourse.masks as masks
import concourse.mybir as mybir
import concourse.tile as tile
from concourse.bass_utils import run_bass_kernel_spmd
from concourse.vector_clock import ScopedClock

F32 = mybir.dt.float32
F32R = mybir.dt.float32r
ALU = mybir.AluOpType
ACTF = mybir.ActivationFunctionType

B, NOBJ, NCOMBO, NTOT, HID, NOUT = 2048, 50, 78, 128, 200, 2
NCORES = 8
BC = B // NCORES  # 256 batch rows per core
EPS = 1e-5
H2 = HID - 128

# matmul operand dtype: float32 (exact, 4cy/row) or float32r (1cy/row, relaxed)
MM_DT = F32R


def _patch_tail_drain():
    """walrus in this container accepts only ONE sync-wait per Drain; Tile's
    tail drain aggregates one wait per active processor.  Split it into a
    chain of single-wait drains."""
    if getattr(tile.TileContext, "_drain_patched", False):
        return

    def _drain_and_barrier(self, tick_clock, wait_clock):
        nc = self.nc
        drain_inst = nc.sync.drain()
        wait_clock.add_sem_waits(
            drain_inst.ins, ScopedClock({None: tick_clock.global_clock})
        )
        si = drain_inst.ins.sync_info
        waits = list(si.on_wait) if si is not None else []
        if len(waits) > 1:
            si.on_wait = waits[:1]
            for w in waits[1:]:
                d2 = nc.sync.drain()
                d2.ins.sync_info = mybir.SyncInfo(on_wait=[w], on_update=[])
        nc.all_engine_barrier()
        assert self.sems is not None
        popped = nc._tile_sem_poison_stack.pop()
        assert popped is self._sem_poison
        nc.clear_and_free_semaphores(list(self.sems.allocated().values()))
        nc.all_engine_barrier()

    tile.TileContext._drain_and_barrier = _drain_and_barrier
    tile.TileContext._drain_patched = True


_WSPLIT_N = [0]


def _split_multi_waits(nc):
    """walrus here accepts only ONE sync-wait per instruction; Tile can emit
    several.  Hoist extras onto same-engine EventSemaphores inserted before."""
    for fn in nc.m.functions:
        for bb in fn.blocks:
            out = []
            changed = False
            for inst in bb.instructions:
                si = inst.sync_info
                waits = list(si.on_wait) if si is not None else []
                if len(waits) > 1:
                    changed = True
                    for w in waits[:-1]:
                        _WSPLIT_N[0] += 1
                        nop = mybir.InstEventSemaphore(
                            name=f"wsplit-{_WSPLIT_N[0]}", ins=[], outs=[]
                        )
                        nop.engine = inst.engine
                        nop.sync_info = mybir.SyncInfo(on_wait=[w], on_update=[])
                        out.append(nop)
                    si.on_wait = waits[-1:]
                out.append(inst)
            if changed:
                bb.instructions = out


def _mm(nc, out, lhsT, rhs, **kw):
    if lhsT.dtype != MM_DT:
        lhsT = lhsT.bitcast(MM_DT)
    if rhs.dtype != MM_DT:
        rhs = rhs.bitcast(MM_DT)
    nc.tensor.matmul(out, lhsT, rhs, **kw)


def _r(ap):
    """Read a (possibly f32r-declared) AP as plain f32 on non-PE engines."""
    return ap.bitcast(F32) if ap.dtype != F32 else ap


# blob_s [50, 640] col layout: acv | aun | aup | ae | apw (128 cols each)
# blob_w [128, 129]: wdt | rw


def build_launch1(iters: int = 1):
    """Per core: vec [BC,200] -> feats [128,5,BC] (comp-major) + stats [128,10]
    (cols 0..4 batch-sums of masses/ptsq/e/wd/pz, 5..9 sums of squares)."""
    _patch_tail_drain()
    nc = bass.Bass(trn_type="TRN2")

    vec_d = nc.dram_tensor("vec", [BC, 4 * NOBJ], F32, kind="ExternalInput")
    blobs_d = nc.dram_tensor("blob_s", [NOBJ, 640], MM_DT, kind="ExternalInput")
    blobw_d = nc.dram_tensor("blob_w", [128, 129], MM_DT, kind="ExternalInput")
    out_d = nc.dram_tensor("out1", [128, 5 * BC + 10], MM_DT, kind="ExternalOutput")

    nblk = BC // 128

    with tile.TileContext(nc) as tc, ExitStack() as ctx:
        consts = ctx.enter_context(tc.tile_pool(name="consts", bufs=1))
        vpool = ctx.enter_context(tc.tile_pool(name="vpool", bufs=2))
        vtpool = ctx.enter_context(tc.tile_pool(name="vtpool", bufs=2))
        sbw = ctx.enter_context(tc.tile_pool(name="sbw", bufs=2))
        work = ctx.enter_context(tc.tile_pool(name="work", bufs=2))
        feats_pool = ctx.enter_context(tc.tile_pool(name="featsp", bufs=2))
        stats_pool = ctx.enter_context(tc.tile_pool(name="statsp", bufs=2))
        pt_ps = ctx.enter_context(tc.tile_pool(name="pt", bufs=2, space="PSUM"))
        mm_ps = ctx.enter_context(tc.tile_pool(name="mm", bufs=5, space="PSUM"))
        w2_ps = ctx.enter_context(tc.tile_pool(name="w2p", bufs=1, space="PSUM"))

        ident_t = consts.tile([128, 128], F32, tag="ident")
        masks.make_identity(nc, ident_t[:])
        ident = ident_t[:]
        blob_s = consts.tile([NOBJ, 640], MM_DT, tag="blob_s")
        nc.scalar.dma_start(blob_s[:], blobs_d[:])
        blob_w = consts.tile([128, 129], MM_DT, tag="blob_w")
        nc.scalar.dma_start(blob_w[:], blobw_d[:])
        wdt = blob_w[:, 0:128]
        rw = _r(blob_w[:, 128:129])
        acv = blob_s[:, 0:128]
        aun = blob_s[:, 128:256]
        aup = blob_s[:, 256:384]
        ae = blob_s[:, 384:512]
        apw = blob_s[:, 512:640]

        lowp = nc.allow_low_precision(reason="stats sums rounded to f32r storage")
        lowp.__enter__()
        for _ in range(iters):
            # ---- one DMA for the batch shard, then 8 PE transposes into
            # component-pair tiles vt01/vt23 [50, 2, BC]
            v2 = vpool.tile([128, nblk, 4 * NOBJ], F32, tag="v2")
            nc.sync.dma_start(
                v2[:], vec_d.rearrange("(blk p) j -> p blk j", blk=nblk)
            )
            vt = []
            for pair in range(2):
                ptp = pt_ps.tile([NOBJ, 2, BC], F32, tag="ptp")
                for half in range(2):
                    i = pair * 2 + half
                    for blk in range(nblk):
                        v3 = v2[:, blk, :].rearrange("p (j c) -> p c j", c=4)
                        nc.tensor.transpose(
                            ptp[:, half, blk * 128 : (blk + 1) * 128],
                            v3[:, i, :],
                            ident,
                        )
                vtp = vtpool.tile([NOBJ, 2, BC], MM_DT, tag=f"vt{pair}", name=f"vt{pair}")
                nc.scalar.copy(vtp[:], ptp[:])
                vt.append(vtp)
            vt01 = vt[0][:].rearrange("j a b -> j (a b)")
            vt23 = vt[1][:].rearrange("j a b -> j (a b)")

            # ---- matmuls (paired, N=512 where possible)
            cv01 = mm_ps.tile([NTOT, 2 * BC], F32, tag="mm")
            _mm(nc, cv01[:], acv, vt01, start=True, stop=True)
            cv23 = mm_ps.tile([NTOT, 2 * BC], F32, tag="mm")
            _mm(nc, cv23[:], acv, vt23, start=True, stop=True)
            u01 = mm_ps.tile([NTOT, 2 * BC], F32, tag="mm")
            _mm(nc, u01[:], aun, vt01, start=True, stop=True)
            u23 = mm_ps.tile([NTOT, 2 * BC], F32, tag="mm")
            _mm(nc, u23[:, 0:BC], aun, vt[1][:, 0, :], start=True, stop=True)
            _mm(nc, u23[:, BC : 2 * BC], aup, vt[1][:, 1, :], start=True, stop=True)
            epz = mm_ps.tile([NTOT, 2 * BC], F32, tag="mm")
            _mm(nc, epz[:, 0:BC], ae, vt[0][:, 0, :], start=True, stop=True)
            _mm(nc, epz[:, BC : 2 * BC], apw, vt[1][:, 1, :], start=True, stop=True)

            # ---- elementwise, spread across ACT / DVE / GPSIMD
            sq01 = sbw.tile([NTOT, 2 * BC], F32, tag="sq01")
            nc.scalar.square(sq01[:], cv01[:])
            sq23 = sbw.tile([NTOT, 2 * BC], F32, tag="sq23")
            nc.scalar.square(sq23[:], cv23[:])

            cvs01 = sbw.tile([NTOT, 2 * BC], F32, tag="cvs01")
            nc.vector.tensor_scalar_mul(cvs01[:], cv01[:], 1.0)
            cvs23 = sbw.tile([NTOT, 2 * BC], F32, tag="cvs23")
            nc.vector.tensor_scalar_mul(cvs23[:], cv23[:], 1.0)

            outb = feats_pool.tile([128, 5 * BC + 16], MM_DT, tag="outb")
            feats = outb[:, 0 : 5 * BC].rearrange("p (k b) -> p k b", k=5)
            stats = outb[:, 5 * BC : 5 * BC + 16]

            # feats comp order: 0 masses, 1 ptsq, 2 e, 3 pz, 4 wd
            # ptsq = sq1 + sq2 ; masses = (sq3 - sq0) - ptsq
            m1 = work.tile([NTOT, BC], F32, tag="m1")
            nc.gpsimd.tensor_tensor(
                m1[:], sq23[:, BC : 2 * BC], sq01[:, 0:BC], op=ALU.subtract
            )
            nc.vector.scalar_tensor_tensor(
                out=feats[:, 1, :], in0=sq01[:, BC : 2 * BC], scalar=1.0,
                in1=sq23[:, 0:BC], op0=ALU.mult, op1=ALU.add,
                accum_out=stats[:, 1:2],
            )
            nc.vector.scalar_tensor_tensor(
                out=feats[:, 0, :], in0=m1[:], scalar=1.0, in1=_r(feats[:, 1, :]),
                op0=ALU.mult, op1=ALU.subtract, accum_out=stats[:, 0:1],
            )

            cm = sbw.tile([NTOT, 4, BC], F32, tag="cm")
            nc.vector.tensor_tensor(
                cm[:, 0:2, :].rearrange("p a b -> p (a b)"), cvs01[:], u01[:],
                op=ALU.mult,
            )
            nc.vector.tensor_tensor(
                cm[:, 2:4, :].rearrange("p a b -> p (a b)"), cvs23[:], u23[:],
                op=ALU.mult,
            )

            nc.scalar.activation(
                feats[:, 2, :], epz[:, 0:BC], ACTF.Copy, accum_out=stats[:, 2:3]
            )
            nc.scalar.activation(
                feats[:, 3, :], epz[:, BC : 2 * BC], ACTF.Copy,
                accum_out=stats[:, 3:4],
            )

            # wd = masses*rw + w_dist@masses - (cm0+cm1) - (cm2+cm3)
            wd2p = w2_ps.tile([NTOT, BC], F32, tag="wd2")
            _mm(nc, wd2p[:], wdt, feats[:, 0, :], start=True, stop=True)
            xa = work.tile([NTOT, 2, BC], F32, tag="xa")
            nc.gpsimd.tensor_tensor(
                xa[:].rearrange("p a b -> p (a b)"),
                cm[:, 0:2, :].rearrange("p a b -> p (a b)"),
                cm[:, 2:4, :].rearrange("p a b -> p (a b)"),
                op=ALU.add,
            )
            x12 = work.tile([NTOT, BC], F32, tag="x12")
            nc.gpsimd.tensor_tensor(x12[:], xa[:, 0, :], xa[:, 1, :], op=ALU.add)
            wd_t = work.tile([NTOT, BC], F32, tag="wd_t")
            nc.vector.scalar_tensor_tensor(
                out=wd_t[:], in0=_r(feats[:, 0, :]), scalar=rw, in1=wd2p[:],
                op0=ALU.mult, op1=ALU.add,
            )
            nc.vector.scalar_tensor_tensor(
                out=feats[:, 4, :], in0=wd_t[:], scalar=1.0, in1=x12[:],
                op0=ALU.mult, op1=ALU.subtract, accum_out=stats[:, 4:5],
            )

            # sums of squares; split ACT / DVE
            for k, eng in ((0, "a"), (1, "v"), (2, "v"), (3, "a"), (4, "v")):
                if eng == "a":
                    scr = work.tile([NTOT, BC], F32, tag="scr_a")
                    nc.scalar.activation(
                        scr[:], _r(feats[:, k, :]), ACTF.Square,
                        accum_out=stats[:, 5 + k : 6 + k],
                    )
                else:
                    scr = work.tile([NTOT, BC], F32, tag="scr_v")
                    nc.vector.scalar_tensor_tensor(
                        out=scr[:], in0=_r(feats[:, k, :]), scalar=1.0,
                        in1=_r(feats[:, k, :]), op0=ALU.mult, op1=ALU.mult,
                        accum_out=stats[:, 5 + k : 6 + k],
                    )

            nc.sync.dma_start(out_d[:, 0 : 4 * BC], outb[:, 0 : 4 * BC])
            nc.scalar.dma_start(
                out_d[:, 4 * BC : 5 * BC + 10], outb[:, 4 * BC : 5 * BC + 10]
            )
        lowp.__exit__(None, None, None)

    _split_multi_waits(nc)
    return nc


# blob2 column layout: per-k [W1a_k | W1b_k] blocks of 200 cols, then consts
_C_W2A, _C_C1A, _C_W2B, _C_C1B, _C_B2, _C_ID2, _C2_END = (
    1000, 1002, 1003, 1005, 1006, 1007, 1009,
)


def build_launch2(iters: int = 1):
    """Per core: featsn [128,5,BC] (BN folded into W1 on host) -> y [BC,2]."""
    _patch_tail_drain()
    nc = bass.Bass(trn_type="TRN2")

    feats_d = nc.dram_tensor("featsn", [128, 5 * BC + 10], MM_DT, kind="ExternalInput")
    blob_d = nc.dram_tensor("blob2", [128, _C2_END], MM_DT, kind="ExternalInput")
    y_d = nc.dram_tensor("y", [BC, NOUT], F32, kind="ExternalOutput")

    nblk = BC // 128

    with tile.TileContext(nc) as tc, ExitStack() as ctx:
        consts = ctx.enter_context(tc.tile_pool(name="consts", bufs=1))
        fpool = ctx.enter_context(tc.tile_pool(name="fpool", bufs=2))
        work = ctx.enter_context(tc.tile_pool(name="work", bufs=2))
        h_ps = ctx.enter_context(tc.tile_pool(name="hps", bufs=2, space="PSUM"))
        o_ps = ctx.enter_context(tc.tile_pool(name="ops", bufs=2, space="PSUM"))
        t_ps = ctx.enter_context(tc.tile_pool(name="tps", bufs=2, space="PSUM"))

        blob = consts.tile([128, _C2_END], MM_DT, tag="blob")
        c1a = _r(blob[:, _C_C1A : _C_C1A + 1])
        c1b = _r(blob[0:H2, _C_C1B : _C_C1B + 1])
        b2t = _r(blob[0:NOUT, _C_B2 : _C_B2 + 1])
        id2 = _r(blob[0:NOUT, _C_ID2:_C2_END])

        first_iter = [True]
        for _ in range(iters):
            nf3 = fpool.tile([128, 5, BC], MM_DT, tag="nf")
            nf = nf3[:]
            nfl = nf3[:].rearrange("p k b -> p (k b)")
            nc.sync.dma_start(nfl[:, 0 : 2 * BC], feats_d[:, 0 : 2 * BC])
            if first_iter[0]:
                nc.scalar.dma_start(blob[:, 0:400], blob_d[:, 0:400])
            nc.sync.dma_start(nfl[:, 2 * BC : 5 * BC], feats_d[:, 2 * BC : 5 * BC])
            if first_iter[0]:
                nc.scalar.dma_start(blob[:, 400:_C2_END], blob_d[:, 400:_C2_END])
                first_iter[0] = False

            ph1 = h_ps.tile([128, BC], F32, tag="ph1")
            ph2 = h_ps.tile([H2, BC], F32, tag="ph2")
            for k in range(5):
                _mm(
                    nc, ph1[:], blob[:, 200 * k : 200 * k + 128],
                    nf[:, k, :], start=(k == 0), stop=(k == 4),
                )
                _mm(
                    nc, ph2[:], blob[:, 200 * k + 128 : 200 * (k + 1)],
                    nf[:, k, :], start=(k == 0), stop=(k == 4),
                )

            hA = work.tile([128, BC], MM_DT, tag="hA")
            nc.scalar.activation(hA[:], ph1[:], ACTF.Relu, bias=c1a)
            hB = work.tile([H2, BC], MM_DT, tag="hB")
            nc.scalar.activation(hB[:], ph2[:], ACTF.Relu, bias=c1b)

            po = o_ps.tile([NOUT, BC], F32, tag="po")
            _mm(nc, po[:], blob[:, _C_W2A : _C_W2A + NOUT], hA[:], start=True,
                stop=False)
            _mm(nc, po[:], blob[0:H2, _C_W2B : _C_W2B + NOUT], hB[:], start=False,
                stop=True)

            so = work.tile([NOUT, BC], F32, tag="so")
            nc.scalar.activation(so[:], po[:], ACTF.Sigmoid, bias=b2t)
            nc.sync.dma_start(y_d.rearrange("b o -> o b"), so[:])

    _split_multi_waits(nc)
    return nc


def _host_prep1(w_combo, w_dist, w_ener, w_pid):
    combo = np.concatenate(
        [np.eye(NOBJ, dtype=np.float32), w_combo.astype(np.float32)], axis=0
    )  # [128, 50]
    a_u = (2.0 * (w_dist @ combo)).T.astype(np.float32)
    bs = np.zeros((NOBJ, 640), np.float32)
    bs[:, 0:128] = combo.T
    bs[:, 128:256] = -a_u
    bs[:, 256:384] = a_u
    bs[:, 384:512] = (w_ener @ combo).T.astype(np.float32)
    bs[:, 512:640] = (w_pid @ combo).T.astype(np.float32)
    bw = np.zeros((128, 129), np.float32)
    bw[:, 0:128] = w_dist.T.astype(np.float32)
    bw[:, 128] = w_dist.sum(axis=1, dtype=np.float32)
    return {"blob_s": bs, "blob_w": bw}


# device comp order k_new: 0 masses, 1 ptsq, 2 w_e, 3 w_pz, 4 w_d;
# reference feature f = 5n + k_orig with k_orig order [m, ptsq, w_e, w_d, w_pz]
_KORIG = [0, 1, 2, 4, 3]
_PERM = np.array(
    [5 * (f % NTOT) + _KORIG[f // NTOT] for f in range(5 * NTOT)], dtype=np.int64
)


def _host_prep2(stats_list, gamma, beta, W1, b1, W2, b2):
    S = np.sum(np.stack(stats_list, 0), axis=0)  # [128, 10]
    S1 = np.ascontiguousarray(S[:, 0:5].T).reshape(5 * NTOT)  # comp-major sums
    S2 = np.ascontiguousarray(S[:, 5:10].T).reshape(5 * NTOT)
    meanp = S1 / B
    varp = S2 / B - meanp * meanp
    gp = gamma[_PERM].astype(np.float32)
    bp = beta[_PERM].astype(np.float32)
    W1p = W1[_PERM, :].astype(np.float32)  # [640, 200]
    a = (gp / np.sqrt(varp + EPS)).astype(np.float32)
    d = (bp - meanp * a).astype(np.float32)
    W1s = (a[:, None] * W1p).astype(np.float32)
    c1 = (W1p.T @ d + b1).astype(np.float32)  # [200]
    W1s3 = W1s.reshape(5, NTOT, HID).transpose(1, 0, 2)  # [128, 5, 200]
    blob = np.zeros((128, _C2_END), np.float32)
    blob[:, 0:1000] = np.ascontiguousarray(W1s3).reshape(128, 1000)
    blob[:, _C_W2A:_C_C1A] = W2[0:128, :].astype(np.float32)
    blob[:, _C_C1A] = c1[0:128]
    blob[0:H2, _C_W2B:_C_C1B] = W2[128:HID, :].astype(np.float32)
    blob[0:H2, _C_C1B] = c1[128:HID]
    blob[0:NOUT, _C_B2] = b2.astype(np.float32)
    blob[0:NOUT, _C_ID2:_C2_END] = np.eye(NOUT, dtype=np.float32)
    return {"blob2": blob}


_CACHE = {}


def _get_kernels(iters: int = 1):
    key = ("k", iters, str(MM_DT))
    if key not in _CACHE:
        _CACHE[key] = (build_launch1(iters), build_launch2(iters))
    return _CACHE[key]


def kernel(vectors, w_combo, w_dist, w_ener, w_pid, gamma, beta, W1, b1, W2, b2):
    vectors = np.asarray(vectors, dtype=np.float32)
    nc1, nc2 = _get_kernels()
    consts1 = _host_prep1(
        np.asarray(w_combo, np.float32),
        np.asarray(w_dist, np.float32),
        np.asarray(w_ener, np.float32),
        np.asarray(w_pid, np.float32),
    )
    in_maps1 = [
        {"vec": np.ascontiguousarray(vectors[c * BC : (c + 1) * BC]), **consts1}
        for c in range(NCORES)
    ]
    r1 = run_bass_kernel_spmd(nc1, in_maps1, core_ids=list(range(NCORES)))
    stats_list = [r1.results[c]["out1"][:, 5 * BC : 5 * BC + 10] for c in range(NCORES)]
    consts2 = _host_prep2(
        stats_list,
        np.asarray(gamma, np.float32),
        np.asarray(beta, np.float32),
        np.asarray(W1, np.float32),
        np.asarray(b1, np.float32),
        np.asarray(W2, np.float32),
        np.asarray(b2, np.float32),
    )
    in_maps2 = [
        {"featsn": r1.results[c]["out1"], **consts2} for c in range(NCORES)
    ]
    r2 = run_bass_kernel_spmd(nc2, in_maps2, core_ids=list(range(NCORES)))
    return np.concatenate([r2.results[c]["y"] for c in range(NCORES)], axis=0)


if __name__ == "__main__":
    np.random.seed(0)
    inputs = {
        "vectors": np.random.randn(B, 4 * NOBJ).astype(np.float32),
        "w_combo": np.random.randn(NCOMBO, NOBJ).astype(np.float32),
        "w_dist": np.random.randn(NTOT, NTOT).astype(np.float32),
        "w_ener": np.random.randn(NTOT, NTOT).astype(np.float32),
        "w_pid": np.random.randn(NTOT, NTOT).astype(np.float32),
        "gamma": np.ones(5 * NTOT, np.float32),
        "beta": np.zeros(5 * NTOT, np.float32),
        "W1": np.random.randn(5 * NTOT, HID).astype(np.float32) / 25.3,
        "b1": np.zeros(HID, np.float32),
        "W2": np.random.randn(HID, NOUT).astype(np.float32) / 14.1,
        "b2": np.zeros(NOUT, np.float32),
    }
    out = kernel(**inputs)
    print("out", out.shape, out.dtype, out[:2])
